# revision 17
# baseline (speedup 1.0000x reference)
"""DetectionLoss kernel for Trainium2 (Bass/Tile), 8-core data parallel.

Problem: B=16 images, P=16384 predicted boxes, T=128 true boxes, C=80 classes.
  bbox_loss = sum(smooth_l1(pred - matched_true) * (max_iou > 0.5)) / max(4*n_matched, 1)
  cls_loss  = -mean over B of log_softmax(pred_classes[:,0,:])[label[:,0]]
  out       = bbox_loss + cls_loss   (f32 scalar)

Sharding: batch dim across 8 cores (2 images per core). Each core returns
per-partition partial sums (bbox sl1 sums, match counts, cls NLL); the host
combines them into the final scalar.

Key algebraic facts used on-device:
  * argmax_t IoU == argmax_t w  where w = inter/(pa+ta), since
    IoU = w/(1-w) is monotone in w; and IoU>0.5 <=> w>1/3.
  * For any matched pair (IoU>0.5) the boxes overlap, so |pred-true| < 1 in
    every coordinate => smooth_l1 branch is always 0.5*d^2 when it matters
    (unmatched rows are masked to zero anyway).
  * One-hot by (w == max_t w): exact-tie multi-hots only occur with
    probability ~0 at w>1/3 (and always at w==0, where mask kills the row).
"""

import numpy as np

import concourse.bacc as bacc
import concourse.bass as bass
import concourse.tile as tile
from concourse import mybir
from concourse.bass_utils import run_bass_kernel_spmd
from concourse.masks import make_identity

F32 = mybir.dt.float32
ALU = mybir.AluOpType
ACTF = mybir.ActivationFunctionType

B, P_TOT, T, C = 16, 16384, 128, 80
NCORES = 8
NIMG = B // NCORES          # images per core
NP = 128                    # partitions
NCH = P_TOT // NP           # chunks per image
THRESH = float(np.log(np.float32(1.0 / 3.0)))  # lw > ln(1/3) <=> IoU > 0.5


def build_nc():
    nc = bacc.Bacc("TRN2", target_bir_lowering=False, debug=False)

    pred_d = nc.declare_dram_parameter("pred", [NIMG, P_TOT, 4], F32, isOutput=False)
    tb_d = nc.declare_dram_parameter("tb", [NIMG, T, 4], F32, isOutput=False)
    tbT_d = nc.declare_dram_parameter("tbT", [NIMG, 4, T], F32, isOutput=False)
    logits_d = nc.declare_dram_parameter("logits", [NIMG, C], F32, isOutput=False)
    oh80_d = nc.declare_dram_parameter("oh80", [NIMG, C], F32, isOutput=False)
    out_d = nc.declare_dram_parameter("out", [NP, 8], F32, isOutput=True)

    with tile.TileContext(nc) as tc:
        consts = tc.alloc_tile_pool(name="consts", bufs=1)
        imgp = tc.alloc_tile_pool(name="imgp", bufs=2)
        chkp = tc.alloc_tile_pool(name="chkp", bufs=3)
        psp = tc.alloc_tile_pool(name="psp", bufs=4, space="PSUM")
        mpsp = tc.alloc_tile_pool(name="mpsp", bufs=2, space="PSUM")

        ident = consts.tile([NP, NP], F32)
        make_identity(nc, ident)

        eps_col = consts.tile([NP, 1], F32)
        nc.vector.memset(eps_col, 1e-38)

        out_sb = consts.tile([NP, 8], F32)
        nc.vector.memset(out_sb, 0.0)

        # ---------------- classification loss (tiny) ----------------
        logit_sb = consts.tile([NIMG, C], F32)
        nc.sync.dma_start(out=logit_sb, in_=logits_d.ap())
        oh_sb = consts.tile([NIMG, C], F32)
        nc.sync.dma_start(out=oh_sb, in_=oh80_d.ap())

        mx = consts.tile([NIMG, 1], F32)
        nc.vector.tensor_reduce(mx, logit_sb, mybir.AxisListType.X, ALU.max)
        zc = consts.tile([NIMG, C], F32)
        nc.vector.tensor_scalar(zc, logit_sb, mx, None, ALU.subtract)
        ez = consts.tile([NIMG, C], F32)
        se = consts.tile([NIMG, 1], F32)
        nc.scalar.activation(ez, zc, ACTF.Exp, accum_out=se)
        lnse = consts.tile([NIMG, 1], F32)
        nc.scalar.activation(lnse, se, ACTF.Ln)
        zl = consts.tile([NIMG, 1], F32)
        zprod = consts.tile([NIMG, C], F32)
        nc.vector.tensor_tensor(zprod, zc, oh_sb, ALU.mult)
        nc.vector.tensor_reduce(zl, zprod, mybir.AxisListType.X, ALU.add)
        # nll = lnse - (z_label - mx) = lse - z_label
        nc.vector.tensor_tensor(out_sb[0:NIMG, 4:5], lnse, zl, ALU.subtract)

        # ---------------- bbox loss ----------------
        for img in range(NIMG):
            # pred laid out [p, n, coord] with row = p*NCH + n: each partition
            # loads 128 consecutive rows => fully contiguous 2KB per partition.
            pred_sb = imgp.tile([NP, NCH, 4], F32, tag="pred")
            pred_img = pred_d.ap()[img].rearrange("(p n) c -> p n c", p=NP)
            nc.sync.dma_start(out=pred_sb, in_=pred_img)

            # true boxes, natural layout for matmul rhs: [t, 4]
            tb_sb = imgp.tile([T, 4], F32, tag="tbsb")
            nc.sync.dma_start(out=tb_sb, in_=tb_d.ap()[img])

            # broadcast tiles: every partition holds the t-row of each coord
            tbT_img = tbT_d.ap()[img]  # [4, T] contiguous rows
            bt = []
            for coord in range(4):
                btile = imgp.tile([NP, T], F32, tag=f"bt{coord}")
                src = bass.AP(
                    tensor=tbT_img.tensor,
                    offset=tbT_img.offset + coord * T,
                    ap=[[0, NP], [1, T]],
                )
                nc.gpsimd.dma_start(out=btile, in_=src)
                bt.append(btile)
            tx1b, ty1b, tx2b, ty2b = bt

            # true areas broadcast tile: (tx2-tx1)*(ty2-ty1)
            tw = imgp.tile([NP, T], F32, tag="tw")
            nc.vector.tensor_tensor(tw, tx2b, tx1b, ALU.subtract)
            th = imgp.tile([NP, T], F32, tag="th")
            nc.vector.tensor_tensor(th, ty2b, ty1b, ALU.subtract)
            taB = imgp.tile([NP, T], F32, tag="taB")
            nc.vector.tensor_tensor(taB, tw, th, ALU.mult)

            # pred areas per chunk: [p, nch]
            pw = imgp.tile([NP, NCH], F32, tag="pw")
            nc.vector.tensor_tensor(
                pw, pred_sb[:, :, 2], pred_sb[:, :, 0], ALU.subtract
            )
            ph = imgp.tile([NP, NCH], F32, tag="ph")
            nc.vector.tensor_tensor(
                ph, pred_sb[:, :, 3], pred_sb[:, :, 1], ALU.subtract
            )
            # clamp pa >= 0: a jittered pred box can invert (x2<x1), making
            # pa<0 — those rows have inter==0 for every t (masked anyway),
            # but a negative pa would make ln(ta+pa) NaN.
            paRaw = imgp.tile([NP, NCH], F32, tag="paRaw")
            nc.vector.tensor_tensor(paRaw, pw, ph, ALU.mult)
            paAll = imgp.tile([NP, NCH], F32, tag="paAll")
            nc.vector.tensor_scalar(paAll, paRaw, 0.0, None, ALU.max)

            maxwAll = imgp.tile([NP, NCH], F32, tag="maxwAll")
            matched_ps = mpsp.tile([NP, NCH, 4], F32, tag="matched")

            for k in range(NCH):
                px1 = pred_sb[:, k, 0:1]
                py1 = pred_sb[:, k, 1:2]
                px2 = pred_sb[:, k, 2:3]
                py2 = pred_sb[:, k, 3:4]

                a_t = chkp.tile([NP, T], F32, tag="a")
                nc.vector.tensor_scalar(a_t, tx2b, px2, None, ALU.min)
                dxn = chkp.tile([NP, T], F32, tag="dxn")
                nc.vector.scalar_tensor_tensor(
                    dxn, tx1b, px1, a_t, ALU.max, ALU.subtract
                )
                b_t = chkp.tile([NP, T], F32, tag="b")
                nc.vector.tensor_scalar(b_t, ty2b, py2, None, ALU.min)
                dyn = chkp.tile([NP, T], F32, tag="dyn")
                nc.vector.scalar_tensor_tensor(
                    dyn, ty1b, py1, b_t, ALU.max, ALU.subtract
                )
                # relu(dx) = max(-dxn, 0)
                rdx = chkp.tile([NP, T], F32, tag="rdx")
                nc.vector.tensor_scalar(rdx, dxn, -1.0, 0.0, ALU.mult, ALU.max)
                rdy = chkp.tile([NP, T], F32, tag="rdy")
                nc.scalar.activation(rdy, dyn, ACTF.Relu, scale=-1.0)

                inter = chkp.tile([NP, T], F32, tag="inter")
                nc.gpsimd.tensor_tensor(inter, rdx, rdy, ALU.mult)

                # log-domain IoU surrogate: lw = ln(inter) - ln(ta + pa).
                # lw is a monotone transform of IoU (per element), so the
                # argmax matches and IoU>0.5 <=> lw > ln(1/3). +1e-38 keeps
                # ln finite when inter == 0 (those rows are masked anyway).
                lns = chkp.tile([NP, T], F32, tag="lns")
                nc.scalar.activation(
                    lns, taB, ACTF.Ln, bias=paAll[:, k : k + 1]
                )
                lni = chkp.tile([NP, T], F32, tag="lni")
                nc.scalar.activation(lni, inter, ACTF.Ln, bias=eps_col[:, 0:1])

                # lw = lni - lns ; maxw = max_t lw
                w_t = chkp.tile([NP, T], F32, tag="w")
                nc.vector.tensor_tensor(w_t, lni, lns, ALU.subtract)
                nc.vector.tensor_reduce(
                    maxwAll[:, k : k + 1], w_t, mybir.AxisListType.X, ALU.max
                )

                oh_t = chkp.tile([NP, T], F32, tag="oh")
                nc.vector.tensor_scalar(
                    oh_t, w_t, maxwAll[:, k : k + 1], None, ALU.is_equal
                )

                ohT_ps = psp.tile([T, NP], F32, tag="ohT")
                nc.tensor.transpose(ohT_ps, oh_t, ident)
                ohT_sb = chkp.tile([T, NP], F32, tag="ohTsb")
                nc.scalar.activation(ohT_sb, ohT_ps, ACTF.Copy)

                nc.tensor.matmul(
                    matched_ps[:, k, :], ohT_sb, tb_sb, start=True, stop=True
                )

            # image tail: d = pred - matched; sl1x2 = sum_c d^2 (x2 of sl1)
            d_t = imgp.tile([NP, NCH, 4], F32, tag="d")
            nc.vector.tensor_tensor(d_t, pred_sb, matched_ps, ALU.subtract)
            dsq = imgp.tile([NP, NCH, 4], F32, tag="dsq")
            nc.vector.tensor_tensor(dsq, d_t, d_t, ALU.mult)
            s01 = imgp.tile([NP, NCH], F32, tag="s01")
            nc.vector.tensor_tensor(s01, dsq[:, :, 0], dsq[:, :, 1], ALU.add)
            s23 = imgp.tile([NP, NCH], F32, tag="s23")
            nc.vector.tensor_tensor(s23, dsq[:, :, 2], dsq[:, :, 3], ALU.add)
            sl1x2 = imgp.tile([NP, NCH], F32, tag="sl1x2")
            nc.vector.tensor_tensor(sl1x2, s01, s23, ALU.add)

            maskAll = imgp.tile([NP, NCH], F32, tag="maskAll")
            nc.vector.tensor_scalar(maskAll, maxwAll, THRESH, None, ALU.is_gt)
            csum = imgp.tile([NP, NCH], F32, tag="csum")
            nc.vector.tensor_tensor(csum, sl1x2, maskAll, ALU.mult)

            nc.vector.tensor_reduce(
                out_sb[:, img : img + 1], csum, mybir.AxisListType.X, ALU.add
            )
            nc.vector.tensor_reduce(
                out_sb[:, 2 + img : 3 + img], maskAll, mybir.AxisListType.X, ALU.add
            )

        nc.sync.dma_start(out=out_d.ap(), in_=out_sb)

        for p in (mpsp, psp, chkp, imgp, consts):
            p.release()

    nc.compile()
    return nc


_NC_CACHE = None


def _get_nc():
    global _NC_CACHE
    if _NC_CACHE is None:
        _NC_CACHE = build_nc()
    return _NC_CACHE


def make_in_maps(pred_bboxes, pred_classes, true_bboxes, true_labels):
    pred_bboxes = np.ascontiguousarray(pred_bboxes, dtype=np.float32)
    true_bboxes = np.ascontiguousarray(true_bboxes, dtype=np.float32)
    logits0 = np.ascontiguousarray(pred_classes[:, 0, :], dtype=np.float32)
    lab0 = np.asarray(true_labels)[:, 0].astype(np.int64)
    oh80 = np.zeros((B, C), dtype=np.float32)
    oh80[np.arange(B), lab0] = 1.0

    in_maps = []
    for c in range(NCORES):
        s = slice(c * NIMG, (c + 1) * NIMG)
        in_maps.append(
            {
                "pred": pred_bboxes[s],
                "tb": true_bboxes[s],
                "tbT": np.ascontiguousarray(
                    true_bboxes[s].transpose(0, 2, 1)
                ),
                "logits": logits0[s],
                "oh80": oh80[s],
            }
        )
    return in_maps


def combine(outs):
    bbox_sum = 0.0
    n_matched = 0.0
    cls_sum = 0.0
    for o in outs:
        o64 = o.astype(np.float64)
        bbox_sum += o64[:, 0:NIMG].sum()
        n_matched += o64[:, NIMG : 2 * NIMG].sum()
        cls_sum += o64[0:NIMG, 4].sum()
    bbox_loss = 0.5 * bbox_sum / max(4.0 * n_matched, 1.0)
    cls_loss = cls_sum / B
    return np.float32(bbox_loss + cls_loss)


def run_device(in_maps, trace=False, **kwargs):
    nc = _get_nc()
    return run_bass_kernel_spmd(
        nc, in_maps, list(range(NCORES)), trace=trace, **kwargs
    )


def kernel(pred_bboxes, pred_classes, true_bboxes, true_labels):
    in_maps = make_in_maps(pred_bboxes, pred_classes, true_bboxes, true_labels)
    res = run_device(in_maps)
    outs = [res.results[i]["out"] for i in range(NCORES)]
    return combine(outs)
